# revision 15
# baseline (speedup 1.0000x reference)
"""Trainium2 Bass kernel for the hybrid attention head (nn_AttentionHead_Hybrid).

Math (per batch):
    norms  n_i = ||x_i||;  xh = x / n
    O      = product of 2016 Givens rotations (built on host, fp32)
    S[i,j] = xh_i . O . xh_j
    A      = S^2 * n_i n_j ;  P = softmax(A / 8)
    V      = x @ Vw^T + Vb
    out    = LayerNorm(P @ V + x) * gamma + beta

Device formulation (per core, 4 batches):
    W      = diag(s) X with s_n = ||x_n||^-1/2 * 8^-1/4   (host-prepped, f16, transposed)
    R      = W O^T W^T          ->  R[j,i]^2 = A[i,j]/8
    E^T[j,i] = exp(R^2)         (bf16)
    Vt     = [X Vw^T + Vb | 1]  (bf16; ones column gives softmax row-sums for free,
                                 bias via ones-row appended to X^T on host)
    OUT^T  = sum_j Vt[j,:]^T E^T[j,:]   in [65, N] psum, then PE-transpose back
    attn   = OUT[:, :64] / OUT[:, 64];  out = LN(attn + x)

Sharding: data-parallel over batch, 4 batches per core on 8 cores; params replicated.
"""

import math

import numpy as np

import concourse.bacc as bacc
import concourse.bass as bass
import concourse.tile as tile
from concourse import bass_utils, mybir

AF = mybir.ActivationFunctionType
ALU = mybir.AluOpType
DT = mybir.dt

B, N, D = 32, 1024, 64
NCORES = 8
BPC = B // NCORES          # batches per core
NT = N // 128              # 128-row tiles per batch

# how many of the 8 per-batch [128,1024] square chunks run on ACT (rest on DVE)
K_ACT_SQ = 4


def _build_orthogonal(phi: np.ndarray, d: int = D) -> np.ndarray:
    """fp32 replica of the reference jax.lax.scan Givens chain."""
    O = np.eye(d, dtype=np.float32)
    ii, jj = np.triu_indices(d, k=1)
    c = np.cos(phi.astype(np.float32))
    s = np.sin(phi.astype(np.float32))
    for k in range(len(phi)):
        i, j = int(ii[k]), int(jj[k])
        ri = O[i].copy()
        rj = O[j].copy()
        O[i] = c[k] * ri + s[k] * rj
        O[j] = -s[k] * ri + c[k] * rj
    return O


def _build_nc(apply_gamma_beta: bool):
    nc = bacc.Bacc("TRN2", target_bir_lowering=False)

    x_t = nc.dram_tensor("x", [BPC, N, D], DT.float32, kind="ExternalInput")
    wt_t = nc.dram_tensor("wt", [BPC, D, N], DT.float16, kind="ExternalInput")
    xo_t = nc.dram_tensor("xo", [BPC, D + 1, N], DT.float16, kind="ExternalInput")
    o_t = nc.dram_tensor("o", [D, D], DT.float16, kind="ExternalInput")
    vwb_t = nc.dram_tensor("vwb", [D + 1, D], DT.float16, kind="ExternalInput")
    id_t = nc.dram_tensor("ident", [128, 128], DT.float32, kind="ExternalInput")
    gb_t = nc.dram_tensor("gb", [2, D], DT.float32, kind="ExternalInput")
    out_t = nc.dram_tensor("out", [BPC, N, D], DT.float32, kind="ExternalOutput")

    with tile.TileContext(nc) as tc:
        with (
            tc.tile_pool(name="const", bufs=1) as constp,
            tc.tile_pool(name="xp", bufs=2) as xp,
            tc.tile_pool(name="wtp", bufs=2) as wtp,
            tc.tile_pool(name="gp", bufs=2) as gp,
            tc.tile_pool(name="ep", bufs=2) as ep,
            tc.tile_pool(name="vp", bufs=2) as vp,
            tc.tile_pool(name="sqp", bufs=4) as sqp,
            tc.tile_pool(name="otp", bufs=2) as otp,
            tc.tile_pool(name="yp", bufs=2) as yp,
            tc.tile_pool(name="statp", bufs=2) as statp,
            tc.tile_pool(name="ps_r", bufs=2, space="PSUM") as ps_r,
            tc.tile_pool(name="ps_ot", bufs=1, space="PSUM") as ps_ot,
            tc.tile_pool(name="ps_small", bufs=2, space="PSUM") as ps_small,
        ):
            o_sb = constp.tile([D, D], DT.float16)
            nc.sync.dma_start(out=o_sb, in_=o_t[:, :])
            vwb_sb = constp.tile([D + 1, D], DT.float16)
            nc.sync.dma_start(out=vwb_sb, in_=vwb_t[:, :])
            id_sb = constp.tile([128, 128], DT.float32)
            nc.sync.dma_start(out=id_sb, in_=id_t[:, :])
            if apply_gamma_beta:
                gam_sb = constp.tile([128, D], DT.float32)
                nc.sync.dma_start(out=gam_sb, in_=gb_t[0, :].to_broadcast([128, D]))
                bet_sb = constp.tile([128, D], DT.float32)
                nc.sync.dma_start(out=bet_sb, in_=gb_t[1, :].to_broadcast([128, D]))

            # PE warm-up: ~12 dependency-free matmuls trip the HAM
            # activity window so real matmuls run at 2.4 GHz, not 1.2.
            warm = constp.tile([64, 512], DT.float16)
            nc.vector.memset(warm, 0.0)
            pw = ps_small.tile([64, 512], DT.float32, tag="sm")
            for _ in range(8):
                nc.tensor.matmul(pw, lhsT=warm[:, 0:64], rhs=warm)

            def emit_loads(b):
                x_sb = xp.tile([128, NT, D], DT.float32, tag="x")
                nc.sync.dma_start(
                    out=x_sb, in_=x_t[b].rearrange("(t p) d -> p t d", p=128)
                )
                wt = wtp.tile([128, N], DT.float16, tag="wt")
                nc.sync.dma_start(out=wt[0:D, :], in_=wt_t[b])
                nc.sync.dma_start(out=wt[D : 2 * D, :], in_=wt_t[b])
                xo = wtp.tile([D + 1, N], DT.float16, tag="xo")
                nc.sync.dma_start(out=xo, in_=xo_t[b])
                return x_sb, wt, xo

            def emit_vg(wt, xo):
                # Vt = [X Vw^T + Vb | 1] bf16 (bias via ones-row in xo)
                pv = ps_small.tile([128, NT, D], DT.float32, tag="sm")
                for t in range(NT):
                    nc.tensor.matmul(
                        pv[:, t, :],
                        lhsT=xo[:, t * 128 : (t + 1) * 128],
                        rhs=vwb_sb,
                    )
                v_sb = vp.tile([128, NT, 66], DT.bfloat16, tag="v")
                nc.vector.tensor_copy(v_sb[:, :, 0:D], pv)
                nc.vector.memset(v_sb[:, :, D], 1.0)
                # G = O^T W^T [64, 1024] f16, duplicated to partitions 64-127
                g_sb = gp.tile([128, N], DT.float16, tag="g")
                for c in range(2):
                    pg = ps_small.tile([D, 512], DT.float32, tag="sm")
                    nc.tensor.matmul(
                        pg, lhsT=o_sb, rhs=wt[0:D, c * 512 : (c + 1) * 512]
                    )
                    nc.scalar.copy(g_sb[0:D, c * 512 : (c + 1) * 512], pg)
                nc.sync.dma_start(out=g_sb[D : 2 * D, :], in_=g_sb[0:D, :])
                return v_sb, g_sb

            state = {0: emit_loads(0)}
            vg = {0: emit_vg(state[0][1], state[0][2])}

            for b in range(BPC):
                x_sb, wt, xo = state[b]
                v_sb, g_sb = vg[b]

                # ---- per j-tile: R, square, exp, PV ----
                e_sb = ep.tile([128, NT, N], DT.bfloat16, tag="e")
                pot = ps_ot.tile([D + 1, N], DT.float32, tag="ot")
                act_set = {(i * NT) // K_ACT_SQ for i in range(K_ACT_SQ)} if K_ACT_SQ else set()
                pwb = ps_small.tile([64, 512], DT.float32, tag="sm")
                prs = {}
                for jt in range(NT):
                    # R pairs: even jt on PE rows 0-63, odd jt on rows 64-127,
                    # issued back-to-back so they run concurrently
                    if jt % 2 == 0:
                        prA = ps_r.tile([128, N], DT.float32, tag="r")
                        prB = ps_r.tile([128, N], DT.float32, tag="r")
                        prs[jt], prs[jt + 1] = prA, prB
                        for c in range(2):
                            nc.tensor.matmul(
                                prA[:, c * 512 : (c + 1) * 512],
                                lhsT=wt[0:D, jt * 128 : (jt + 1) * 128],
                                rhs=g_sb[0:D, c * 512 : (c + 1) * 512],
                                tile_position=(0, 0),
                            )
                            nc.tensor.matmul(
                                prB[:, c * 512 : (c + 1) * 512],
                                lhsT=wt[D : 2 * D, (jt + 1) * 128 : (jt + 2) * 128],
                                rhs=g_sb[D : 2 * D, c * 512 : (c + 1) * 512],
                                tile_position=(64, 0),
                            )
                    pr = prs[jt]
                    nc.tensor.matmul(pwb, lhsT=warm[:, 0:64], rhs=warm)
                    asq = sqp.tile([128, N], DT.float16, tag="asq")
                    if jt in act_set:
                        nc.scalar.activation(asq, pr, AF.Square)
                    else:
                        rf = sqp.tile([128, N], DT.float16, tag="rf")
                        nc.vector.tensor_copy(rf, pr)
                        nc.vector.tensor_mul(asq, rf, rf)
                    nc.scalar.activation(e_sb[:, jt, :], asq, AF.Exp)
                    # PV: OUT^T[65, :] += Vt[jt]^T @ E^T[jt]
                    for c in range(2):
                        nc.tensor.matmul(
                            pot[:, c * 512 : (c + 1) * 512],
                            lhsT=v_sb[:, jt, 0 : D + 1],
                            rhs=e_sb[:, jt, c * 512 : (c + 1) * 512],
                            start=(jt == 0),
                            stop=(jt == NT - 1),
                        )
                    if jt == 2 and b + 1 < BPC:
                        state[b + 1] = emit_loads(b + 1)
                    if jt == 4 and b + 1 < BPC:
                        vg[b + 1] = emit_vg(state[b + 1][1], state[b + 1][2])

                # ---- OUT^T -> SBUF (half DVE, half ACT) ----
                ot_sb = otp.tile([D + 1, N], DT.float32, tag="ot")
                nc.scalar.copy(ot_sb[:, 0:512], pot[:, 0:512])
                nc.scalar.copy(ot_sb[:, 512:N], pot[:, 512:N])

                # ---- transpose back in groups of 4 i-tiles + epilogue ----
                y_sb = yp.tile([128, NT, D], DT.float32, tag="y")
                mean = statp.tile([128, NT], DT.float32, tag="mean")
                var = statp.tile([128, NT], DT.float32, tag="var")
                rstd = statp.tile([128, NT], DT.float32, tag="rstd")
                rcol = statp.tile([128, NT], DT.float32, tag="rcol")
                ysq = statp.tile([128, 4, D], DT.float32, tag="ysq")
                for grp in range(2):
                    ptr = ps_small.tile([128, 4, D + 1], DT.float32, tag="sm")
                    for q in range(4):
                        it = grp * 4 + q
                        nc.tensor.transpose(
                            ptr[:, q, :],
                            ot_sb[:, it * 128 : (it + 1) * 128],
                            id_sb[0 : D + 1, 0 : D + 1],
                        )
                    g_sl = slice(grp * 4, grp * 4 + 4)
                    # 1/rowsum for the 4 tiles at once
                    nc.vector.reciprocal(rcol[:, g_sl], ptr[:, :, D])
                    # y = OUT * (1/rowsum) + x, fused per i-tile
                    for q in range(4):
                        it = grp * 4 + q
                        nc.vector.scalar_tensor_tensor(
                            out=y_sb[:, it, :],
                            in0=ptr[:, q, 0:D],
                            scalar=rcol[:, it : it + 1],
                            in1=x_sb[:, it, :],
                            op0=ALU.mult,
                            op1=ALU.add,
                        )
                    # LN stats via reduces
                    nc.vector.reduce_sum(
                        mean[:, g_sl], y_sb[:, g_sl, :], axis=mybir.AxisListType.X
                    )
                    nc.vector.tensor_mul(ysq, y_sb[:, g_sl, :], y_sb[:, g_sl, :])
                    nc.vector.reduce_sum(
                        var[:, g_sl], ysq, axis=mybir.AxisListType.X
                    )

                # mean/=64; var = var/64 - mean^2 + eps; rstd = rsqrt(var) (Quake+Newton)
                nc.vector.tensor_scalar_mul(mean, mean, 1.0 / D)
                nc.vector.tensor_scalar(
                    var, var, scalar1=1.0 / D, scalar2=1e-5,
                    op0=ALU.mult, op1=ALU.add,
                )
                msq = statp.tile([128, NT], DT.float32, tag="msq")
                nc.gpsimd.tensor_mul(msq, mean, mean)
                nc.gpsimd.tensor_sub(var, var, msq)
                # Quake seed: iv = magic - (i >> 1) == (~(i >> 1)) + (magic + 1)
                iv = statp.tile([128, NT], DT.int32, tag="iv")
                nc.vector.tensor_scalar(
                    iv, var.bitcast(DT.int32), scalar1=1, scalar2=None,
                    op0=ALU.arith_shift_right,
                )
                nc.vector.tensor_scalar(
                    iv, iv, scalar1=-1, scalar2=None, op0=ALU.bitwise_xor
                )
                nc.vector.tensor_scalar_add(iv, iv, 0x5F3759E0)
                yk = iv.bitcast(DT.float32)
                # two Newton steps: y = y * (1.5 - 0.5 v y^2)
                t1 = statp.tile([128, NT], DT.float32, tag="t1")
                for _ in range(2):
                    nc.gpsimd.tensor_mul(t1, yk, yk)
                    nc.gpsimd.tensor_mul(t1, t1, var)
                    nc.vector.tensor_scalar(
                        t1, t1, scalar1=-0.5, scalar2=1.5,
                        op0=ALU.mult, op1=ALU.add,
                    )
                    nc.gpsimd.tensor_mul(yk, yk, t1)
                nc.gpsimd.tensor_copy(rstd, yk)

                for it in range(NT):
                    nc.vector.tensor_scalar(
                        y_sb[:, it, :],
                        y_sb[:, it, :],
                        scalar1=mean[:, it : it + 1],
                        scalar2=rstd[:, it : it + 1],
                        op0=ALU.subtract,
                        op1=ALU.mult,
                    )
                    if apply_gamma_beta:
                        nc.gpsimd.tensor_mul(y_sb[:, it, :], y_sb[:, it, :], gam_sb)
                        nc.gpsimd.tensor_add(y_sb[:, it, :], y_sb[:, it, :], bet_sb)

                nc.sync.dma_start(
                    out=out_t[b].rearrange("(t p) d -> p t d", p=128), in_=y_sb
                )

    nc.compile()
    return nc


_NC_CACHE: dict = {}


def kernel(input1, V_w, V_b, phi, ln_gamma, ln_beta, _trace=False):
    input1 = np.ascontiguousarray(np.asarray(input1, dtype=np.float32))
    V_w = np.asarray(V_w, dtype=np.float32)
    V_b = np.asarray(V_b, dtype=np.float32)
    phi = np.asarray(phi, dtype=np.float32)
    ln_gamma = np.asarray(ln_gamma, dtype=np.float32)
    ln_beta = np.asarray(ln_beta, dtype=np.float32)

    apply_gb = not (np.all(ln_gamma == 1.0) and np.all(ln_beta == 0.0))

    if apply_gb not in _NC_CACHE:
        _NC_CACHE[apply_gb] = _build_nc(apply_gb)
    nc = _NC_CACHE[apply_gb]

    O = _build_orthogonal(phi)
    o16 = np.ascontiguousarray(O.astype(np.float16))
    vwb = np.ascontiguousarray(
        np.concatenate([V_w.T, V_b[None, :]], axis=0).astype(np.float16)
    )
    ident = np.eye(128, dtype=np.float32)
    gb = np.ascontiguousarray(np.stack([ln_gamma, ln_beta]).astype(np.float32))

    # host prep: scaled rows W = x * ||x||^-1/2 * 8^-1/4, transposed; raw X^T + ones row
    ss = (input1.astype(np.float64) ** 2).sum(-1)
    s = (ss ** -0.25 * 8.0 ** -0.25).astype(np.float32)
    w = input1 * s[..., None]
    wt_full = np.ascontiguousarray(w.transpose(0, 2, 1).astype(np.float16))
    xt = input1.transpose(0, 2, 1).astype(np.float16)
    xo_full = np.ascontiguousarray(
        np.concatenate([xt, np.ones((B, 1, N), np.float16)], axis=1)
    )

    in_maps = []
    for c in range(NCORES):
        sl = slice(c * BPC, (c + 1) * BPC)
        in_maps.append(
            {
                "x": np.ascontiguousarray(input1[sl]),
                "wt": wt_full[sl],
                "xo": xo_full[sl],
                "o": o16,
                "vwb": vwb,
                "ident": ident,
                "gb": gb,
            }
        )

    res = bass_utils.run_bass_kernel_spmd(
        nc, in_maps, core_ids=list(range(NCORES)), trace=_trace
    )
    out = np.concatenate([res.results[c]["out"] for c in range(NCORES)], axis=0)
    if _trace:
        kernel._last_result = res
    return out


# revision 16
# speedup vs baseline: 1.2060x; 1.2060x over previous
"""Trainium2 Bass kernel for the hybrid attention head (nn_AttentionHead_Hybrid).

Math (per batch):
    norms  n_i = ||x_i||;  xh = x / n
    O      = product of 2016 Givens rotations (built on host, fp32)
    S[i,j] = xh_i . O . xh_j
    A      = S^2 * n_i n_j ;  P = softmax(A / 8)
    V      = x @ Vw^T + Vb
    out    = LayerNorm(P @ V + x) * gamma + beta

Device formulation (per core, 4 batches):
    W      = diag(s) X with s_n = ||x_n||^-1/2 * 8^-1/4   (host-prepped, f16, transposed)
    R      = W O^T W^T          ->  R[j,i]^2 = A[i,j]/8
    E^T[j,i] = exp(R^2)         (bf16)
    Vt     = [X Vw^T + Vb | 1]  (bf16; ones column gives softmax row-sums for free,
                                 bias via ones-row appended to X^T on host)
    OUT^T  = sum_j Vt[j,:]^T E^T[j,:]   in [65, N] psum, then PE-transpose back
    attn   = OUT[:, :64] / OUT[:, 64];  out = LN(attn + x)

Sharding: data-parallel over batch, 4 batches per core on 8 cores; params replicated.
"""

import math

import numpy as np

import concourse.bacc as bacc
import concourse.bass as bass
import concourse.tile as tile
from concourse import bass_utils, mybir

AF = mybir.ActivationFunctionType
ALU = mybir.AluOpType
DT = mybir.dt

B, N, D = 32, 1024, 64
NCORES = 8
BPC = B // NCORES          # batches per core
NT = N // 128              # 128-row tiles per batch

# how many of the 8 per-batch [128,1024] square chunks run on ACT (rest on DVE)
K_ACT_SQ = 4


def _build_orthogonal(phi: np.ndarray, d: int = D) -> np.ndarray:
    """fp32 replica of the reference jax.lax.scan Givens chain."""
    O = np.eye(d, dtype=np.float32)
    ii, jj = np.triu_indices(d, k=1)
    c = np.cos(phi.astype(np.float32))
    s = np.sin(phi.astype(np.float32))
    for k in range(len(phi)):
        i, j = int(ii[k]), int(jj[k])
        ri = O[i].copy()
        rj = O[j].copy()
        O[i] = c[k] * ri + s[k] * rj
        O[j] = -s[k] * ri + c[k] * rj
    return O


def _build_nc(apply_gamma_beta: bool):
    nc = bacc.Bacc("TRN2", target_bir_lowering=False)

    x_t = nc.dram_tensor("x", [BPC, N, D], DT.float32, kind="ExternalInput")
    wt_t = nc.dram_tensor("wt", [BPC, D, N], DT.float16, kind="ExternalInput")
    xo_t = nc.dram_tensor("xo", [BPC, D + 1, N], DT.float16, kind="ExternalInput")
    o_t = nc.dram_tensor("o", [D, D], DT.float16, kind="ExternalInput")
    vwb_t = nc.dram_tensor("vwb", [D + 1, D], DT.float16, kind="ExternalInput")
    id_t = nc.dram_tensor("ident", [128, 128], DT.float32, kind="ExternalInput")
    gb_t = nc.dram_tensor("gb", [2, D], DT.float32, kind="ExternalInput")
    out_t = nc.dram_tensor("out", [BPC, N, D], DT.float32, kind="ExternalOutput")

    with tile.TileContext(nc) as tc:
        with (
            tc.tile_pool(name="const", bufs=1) as constp,
            tc.tile_pool(name="xp", bufs=2) as xp,
            tc.tile_pool(name="wtp", bufs=2) as wtp,
            tc.tile_pool(name="gp", bufs=2) as gp,
            tc.tile_pool(name="ep", bufs=2) as ep,
            tc.tile_pool(name="vp", bufs=2) as vp,
            tc.tile_pool(name="sqp", bufs=4) as sqp,
            tc.tile_pool(name="otp", bufs=2) as otp,
            tc.tile_pool(name="yp", bufs=2) as yp,
            tc.tile_pool(name="statp", bufs=2) as statp,
            tc.tile_pool(name="ps_r", bufs=2, space="PSUM") as ps_r,
            tc.tile_pool(name="ps_ot", bufs=1, space="PSUM") as ps_ot,
            tc.tile_pool(name="ps_small", bufs=2, space="PSUM") as ps_small,
        ):
            o_sb = constp.tile([D, D], DT.float16)
            nc.sync.dma_start(out=o_sb, in_=o_t[:, :])
            vwb_sb = constp.tile([D + 1, D], DT.float16)
            nc.sync.dma_start(out=vwb_sb, in_=vwb_t[:, :])
            id_sb = constp.tile([128, 128], DT.float32)
            nc.sync.dma_start(out=id_sb, in_=id_t[:, :])
            if apply_gamma_beta:
                gam_sb = constp.tile([128, D], DT.float32)
                nc.sync.dma_start(out=gam_sb, in_=gb_t[0, :].to_broadcast([128, D]))
                bet_sb = constp.tile([128, D], DT.float32)
                nc.sync.dma_start(out=bet_sb, in_=gb_t[1, :].to_broadcast([128, D]))

            # PE warm-up: ~12 dependency-free matmuls trip the HAM
            # activity window so real matmuls run at 2.4 GHz, not 1.2.
            warm = constp.tile([64, 512], DT.float16)
            nc.vector.memset(warm, 0.0)
            pw = ps_small.tile([64, 512], DT.float32, tag="sm")
            for _ in range(8):
                nc.tensor.matmul(pw, lhsT=warm[:, 0:64], rhs=warm)

            def emit_loads(b):
                x_sb = xp.tile([128, NT, D], DT.float32, tag="x")
                nc.sync.dma_start(
                    out=x_sb, in_=x_t[b].rearrange("(t p) d -> p t d", p=128)
                )
                wt = wtp.tile([128, N], DT.float16, tag="wt")
                nc.sync.dma_start(out=wt[0:D, :], in_=wt_t[b])
                nc.sync.dma_start(out=wt[D : 2 * D, :], in_=wt_t[b])
                xo = wtp.tile([D + 1, N], DT.float16, tag="xo")
                nc.sync.dma_start(out=xo, in_=xo_t[b])
                return x_sb, wt, xo

            def emit_vg(wt, xo):
                # Vt = [X Vw^T + Vb | 1] bf16 (bias via ones-row in xo)
                pv = ps_small.tile([128, NT, D], DT.float32, tag="sm")
                for t in range(NT):
                    nc.tensor.matmul(
                        pv[:, t, :],
                        lhsT=xo[:, t * 128 : (t + 1) * 128],
                        rhs=vwb_sb,
                    )
                v_sb = vp.tile([128, NT, 66], DT.bfloat16, tag="v")
                nc.vector.tensor_copy(v_sb[:, :, 0:D], pv)
                nc.vector.memset(v_sb[:, :, D], 1.0)
                # G = O^T W^T [64, 1024] f16, duplicated to partitions 64-127
                g_sb = gp.tile([128, N], DT.float16, tag="g")
                for c in range(2):
                    pg = ps_small.tile([D, 512], DT.float32, tag="sm")
                    nc.tensor.matmul(
                        pg, lhsT=o_sb, rhs=wt[0:D, c * 512 : (c + 1) * 512]
                    )
                    nc.scalar.copy(g_sb[0:D, c * 512 : (c + 1) * 512], pg)
                nc.sync.dma_start(out=g_sb[D : 2 * D, :], in_=g_sb[0:D, :])
                return v_sb, g_sb

            state = {0: emit_loads(0)}
            vg = {0: emit_vg(state[0][1], state[0][2])}

            for b in range(BPC):
                x_sb, wt, xo = state[b]
                v_sb, g_sb = vg[b]

                # ---- per j-tile: R, square, exp, PV ----
                e_sb = ep.tile([128, NT, N], DT.bfloat16, tag="e")
                pot = ps_ot.tile([D + 1, N], DT.float32, tag="ot")
                act_set = {(i * NT) // K_ACT_SQ for i in range(K_ACT_SQ)} if K_ACT_SQ else set()
                for jt in range(NT):
                    pr = ps_r.tile([128, N], DT.float32, tag="r")
                    for c in range(2):
                        nc.tensor.matmul(
                            pr[:, c * 512 : (c + 1) * 512],
                            lhsT=wt[0:D, jt * 128 : (jt + 1) * 128],
                            rhs=g_sb[0:D, c * 512 : (c + 1) * 512],
                        )
                    asq = sqp.tile([128, N], DT.float16, tag="asq")
                    if jt in act_set:
                        nc.scalar.activation(asq, pr, AF.Square)
                    else:
                        rf = sqp.tile([128, N], DT.float16, tag="rf")
                        nc.vector.tensor_copy(rf, pr)
                        nc.vector.tensor_mul(asq, rf, rf)
                    nc.scalar.activation(e_sb[:, jt, :], asq, AF.Exp)
                    # PV: OUT^T[65, :] += Vt[jt]^T @ E^T[jt]
                    for c in range(2):
                        nc.tensor.matmul(
                            pot[:, c * 512 : (c + 1) * 512],
                            lhsT=v_sb[:, jt, 0 : D + 1],
                            rhs=e_sb[:, jt, c * 512 : (c + 1) * 512],
                            start=(jt == 0),
                            stop=(jt == NT - 1),
                        )
                    if jt == 2 and b + 1 < BPC:
                        state[b + 1] = emit_loads(b + 1)
                    if jt == 4 and b + 1 < BPC:
                        vg[b + 1] = emit_vg(state[b + 1][1], state[b + 1][2])

                # ---- OUT^T -> SBUF (half DVE, half ACT) ----
                ot_sb = otp.tile([D + 1, N], DT.float32, tag="ot")
                nc.vector.tensor_copy(ot_sb[:, 0:512], pot[:, 0:512])
                nc.scalar.copy(ot_sb[:, 512:N], pot[:, 512:N])

                # ---- transpose back in groups of 4 i-tiles + epilogue ----
                y_sb = yp.tile([128, NT, D], DT.float32, tag="y")
                mean = statp.tile([128, NT], DT.float32, tag="mean")
                var = statp.tile([128, NT], DT.float32, tag="var")
                rstd = statp.tile([128, NT], DT.float32, tag="rstd")
                rcol = statp.tile([128, NT], DT.float32, tag="rcol")
                ysq = statp.tile([128, 4, D], DT.float32, tag="ysq")
                for grp in range(2):
                    ptr = ps_small.tile([128, 4, D + 1], DT.float32, tag="sm")
                    for q in range(4):
                        it = grp * 4 + q
                        nc.tensor.transpose(
                            ptr[:, q, :],
                            ot_sb[:, it * 128 : (it + 1) * 128],
                            id_sb[0 : D + 1, 0 : D + 1],
                        )
                    g_sl = slice(grp * 4, grp * 4 + 4)
                    # 1/rowsum for the 4 tiles at once
                    nc.vector.reciprocal(rcol[:, g_sl], ptr[:, :, D])
                    # y = OUT * (1/rowsum) + x, fused per i-tile
                    for q in range(4):
                        it = grp * 4 + q
                        nc.vector.scalar_tensor_tensor(
                            out=y_sb[:, it, :],
                            in0=ptr[:, q, 0:D],
                            scalar=rcol[:, it : it + 1],
                            in1=x_sb[:, it, :],
                            op0=ALU.mult,
                            op1=ALU.add,
                        )
                    # LN stats via reduces
                    nc.vector.reduce_sum(
                        mean[:, g_sl], y_sb[:, g_sl, :], axis=mybir.AxisListType.X
                    )
                    nc.vector.tensor_mul(ysq, y_sb[:, g_sl, :], y_sb[:, g_sl, :])
                    nc.vector.reduce_sum(
                        var[:, g_sl], ysq, axis=mybir.AxisListType.X
                    )

                # mean/=64; var = var/64 - mean^2 + eps; rstd = rsqrt(var) (Quake+Newton)
                nc.vector.tensor_scalar_mul(mean, mean, 1.0 / D)
                nc.vector.tensor_scalar(
                    var, var, scalar1=1.0 / D, scalar2=1e-5,
                    op0=ALU.mult, op1=ALU.add,
                )
                msq = statp.tile([128, NT], DT.float32, tag="msq")
                nc.gpsimd.tensor_mul(msq, mean, mean)
                nc.gpsimd.tensor_sub(var, var, msq)
                # Quake seed: iv = magic - (i >> 1) == (~(i >> 1)) + (magic + 1)
                iv = statp.tile([128, NT], DT.int32, tag="iv")
                nc.vector.tensor_scalar(
                    iv, var.bitcast(DT.int32), scalar1=1, scalar2=None,
                    op0=ALU.arith_shift_right,
                )
                nc.vector.tensor_scalar(
                    iv, iv, scalar1=-1, scalar2=None, op0=ALU.bitwise_xor
                )
                nc.vector.tensor_scalar_add(iv, iv, 0x5F3759E0)
                yk = iv.bitcast(DT.float32)
                # two Newton steps: y = y * (1.5 - 0.5 v y^2)
                t1 = statp.tile([128, NT], DT.float32, tag="t1")
                for _ in range(2):
                    nc.gpsimd.tensor_mul(t1, yk, yk)
                    nc.gpsimd.tensor_mul(t1, t1, var)
                    nc.vector.tensor_scalar(
                        t1, t1, scalar1=-0.5, scalar2=1.5,
                        op0=ALU.mult, op1=ALU.add,
                    )
                    nc.gpsimd.tensor_mul(yk, yk, t1)
                nc.gpsimd.tensor_copy(rstd, yk)

                for it in range(NT):
                    nc.vector.tensor_scalar(
                        y_sb[:, it, :],
                        y_sb[:, it, :],
                        scalar1=mean[:, it : it + 1],
                        scalar2=rstd[:, it : it + 1],
                        op0=ALU.subtract,
                        op1=ALU.mult,
                    )
                    if apply_gamma_beta:
                        nc.gpsimd.tensor_mul(y_sb[:, it, :], y_sb[:, it, :], gam_sb)
                        nc.gpsimd.tensor_add(y_sb[:, it, :], y_sb[:, it, :], bet_sb)

                nc.sync.dma_start(
                    out=out_t[b].rearrange("(t p) d -> p t d", p=128), in_=y_sb
                )

    nc.compile()
    return nc


_NC_CACHE: dict = {}


def kernel(input1, V_w, V_b, phi, ln_gamma, ln_beta, _trace=False):
    input1 = np.ascontiguousarray(np.asarray(input1, dtype=np.float32))
    V_w = np.asarray(V_w, dtype=np.float32)
    V_b = np.asarray(V_b, dtype=np.float32)
    phi = np.asarray(phi, dtype=np.float32)
    ln_gamma = np.asarray(ln_gamma, dtype=np.float32)
    ln_beta = np.asarray(ln_beta, dtype=np.float32)

    apply_gb = not (np.all(ln_gamma == 1.0) and np.all(ln_beta == 0.0))

    if apply_gb not in _NC_CACHE:
        _NC_CACHE[apply_gb] = _build_nc(apply_gb)
    nc = _NC_CACHE[apply_gb]

    O = _build_orthogonal(phi)
    o16 = np.ascontiguousarray(O.astype(np.float16))
    vwb = np.ascontiguousarray(
        np.concatenate([V_w.T, V_b[None, :]], axis=0).astype(np.float16)
    )
    ident = np.eye(128, dtype=np.float32)
    gb = np.ascontiguousarray(np.stack([ln_gamma, ln_beta]).astype(np.float32))

    # host prep: scaled rows W = x * ||x||^-1/2 * 8^-1/4, transposed; raw X^T + ones row
    ss = (input1.astype(np.float64) ** 2).sum(-1)
    s = (ss ** -0.25 * 8.0 ** -0.25).astype(np.float32)
    w = input1 * s[..., None]
    wt_full = np.ascontiguousarray(w.transpose(0, 2, 1).astype(np.float16))
    xt = input1.transpose(0, 2, 1).astype(np.float16)
    xo_full = np.ascontiguousarray(
        np.concatenate([xt, np.ones((B, 1, N), np.float16)], axis=1)
    )

    in_maps = []
    for c in range(NCORES):
        sl = slice(c * BPC, (c + 1) * BPC)
        in_maps.append(
            {
                "x": np.ascontiguousarray(input1[sl]),
                "wt": wt_full[sl],
                "xo": xo_full[sl],
                "o": o16,
                "vwb": vwb,
                "ident": ident,
                "gb": gb,
            }
        )

    res = bass_utils.run_bass_kernel_spmd(
        nc, in_maps, core_ids=list(range(NCORES)), trace=_trace
    )
    out = np.concatenate([res.results[c]["out"] for c in range(NCORES)], axis=0)
    if _trace:
        kernel._last_result = res
    return out


# revision 17
# speedup vs baseline: 1.2749x; 1.0571x over previous
"""Trainium2 Bass kernel for the hybrid attention head (nn_AttentionHead_Hybrid).

Math (per batch):
    norms  n_i = ||x_i||;  xh = x / n
    O      = product of 2016 Givens rotations (built on host, fp32)
    S[i,j] = xh_i . O . xh_j
    A      = S^2 * n_i n_j ;  P = softmax(A / 8)
    V      = x @ Vw^T + Vb
    out    = LayerNorm(P @ V + x) * gamma + beta

Device formulation (per core, 4 batches):
    W      = diag(s) X with s_n = ||x_n||^-1/2 * 8^-1/4   (host-prepped, f16, transposed)
    R      = W O^T W^T          ->  R[j,i]^2 = A[i,j]/8
    E^T[j,i] = exp(R^2)         (bf16)
    Vt     = [X Vw^T + Vb | 1]  (bf16; ones column gives softmax row-sums for free,
                                 bias via ones-row appended to X^T on host)
    OUT^T  = sum_j Vt[j,:]^T E^T[j,:]   in [65, N] psum, then PE-transpose back
    attn   = OUT[:, :64] / OUT[:, 64];  out = LN(attn + x)

Sharding: data-parallel over batch, 4 batches per core on 8 cores; params replicated.
"""

import math

import numpy as np

import concourse.bacc as bacc
import concourse.bass as bass
import concourse.tile as tile
from concourse import bass_utils, mybir

AF = mybir.ActivationFunctionType
ALU = mybir.AluOpType
DT = mybir.dt

B, N, D = 32, 1024, 64
NCORES = 8
BPC = B // NCORES          # batches per core
NT = N // 128              # 128-row tiles per batch

# how many of the 8 per-batch [128,1024] square chunks run on ACT (rest on DVE)
K_ACT_SQ = 4


def _build_orthogonal(phi: np.ndarray, d: int = D) -> np.ndarray:
    """fp32 replica of the reference jax.lax.scan Givens chain."""
    O = np.eye(d, dtype=np.float32)
    ii, jj = np.triu_indices(d, k=1)
    c = np.cos(phi.astype(np.float32))
    s = np.sin(phi.astype(np.float32))
    for k in range(len(phi)):
        i, j = int(ii[k]), int(jj[k])
        ri = O[i].copy()
        rj = O[j].copy()
        O[i] = c[k] * ri + s[k] * rj
        O[j] = -s[k] * ri + c[k] * rj
    return O


def _build_nc(apply_gamma_beta: bool):
    nc = bacc.Bacc("TRN2", target_bir_lowering=False)

    x_t = nc.dram_tensor("x", [BPC, N, D], DT.float32, kind="ExternalInput")
    wt_t = nc.dram_tensor("wt", [BPC, D, N], DT.float16, kind="ExternalInput")
    xo_t = nc.dram_tensor("xo", [BPC, D + 1, N], DT.float16, kind="ExternalInput")
    o_t = nc.dram_tensor("o", [D, D], DT.float16, kind="ExternalInput")
    vwb_t = nc.dram_tensor("vwb", [D + 1, D], DT.float16, kind="ExternalInput")
    id_t = nc.dram_tensor("ident", [128, 128], DT.float32, kind="ExternalInput")
    gb_t = nc.dram_tensor("gb", [2, D], DT.float32, kind="ExternalInput")
    out_t = nc.dram_tensor("out", [BPC, N, D], DT.float32, kind="ExternalOutput")

    with tile.TileContext(nc) as tc:
        with (
            tc.tile_pool(name="const", bufs=1) as constp,
            tc.tile_pool(name="xp", bufs=2) as xp,
            tc.tile_pool(name="wtp", bufs=2) as wtp,
            tc.tile_pool(name="gp", bufs=2) as gp,
            tc.tile_pool(name="ep", bufs=2) as ep,
            tc.tile_pool(name="vp", bufs=2) as vp,
            tc.tile_pool(name="sqp", bufs=4) as sqp,
            tc.tile_pool(name="otp", bufs=2) as otp,
            tc.tile_pool(name="yp", bufs=2) as yp,
            tc.tile_pool(name="statp", bufs=2) as statp,
            tc.tile_pool(name="ps_r", bufs=2, space="PSUM") as ps_r,
            tc.tile_pool(name="ps_ot", bufs=1, space="PSUM") as ps_ot,
            tc.tile_pool(name="ps_small", bufs=2, space="PSUM") as ps_small,
        ):
            o_sb = constp.tile([128, D], DT.float16)
            nc.sync.dma_start(out=o_sb[0:D, :], in_=o_t[:, :])
            nc.sync.dma_start(out=o_sb[D : 2 * D, :], in_=o_t[:, :])
            vwb_sb = constp.tile([D + 1, D], DT.float16)
            nc.sync.dma_start(out=vwb_sb, in_=vwb_t[:, :])
            id_sb = constp.tile([128, 128], DT.float32)
            nc.sync.dma_start(out=id_sb, in_=id_t[:, :])
            if apply_gamma_beta:
                gam_sb = constp.tile([128, D], DT.float32)
                nc.sync.dma_start(out=gam_sb, in_=gb_t[0, :].to_broadcast([128, D]))
                bet_sb = constp.tile([128, D], DT.float32)
                nc.sync.dma_start(out=bet_sb, in_=gb_t[1, :].to_broadcast([128, D]))

            # PE warm-up: ~12 dependency-free matmuls trip the HAM
            # activity window so real matmuls run at 2.4 GHz, not 1.2.
            warm = constp.tile([64, 512], DT.float16)
            nc.vector.memset(warm, 0.0)
            pw = ps_small.tile([64, 512], DT.float32, tag="sm")
            for _ in range(8):
                nc.tensor.matmul(pw, lhsT=warm[:, 0:64], rhs=warm)

            def emit_loads(b):
                x_sb = xp.tile([128, NT, D], DT.float32, tag="x")
                nc.sync.dma_start(
                    out=x_sb, in_=x_t[b].rearrange("(t p) d -> p t d", p=128)
                )
                wt = wtp.tile([128, N], DT.float16, tag="wt")
                nc.sync.dma_start(out=wt[0:D, :], in_=wt_t[b])
                nc.sync.dma_start(out=wt[D : 2 * D, :], in_=wt_t[b])
                xo = wtp.tile([D + 1, N], DT.float16, tag="xo")
                nc.sync.dma_start(out=xo, in_=xo_t[b])
                return x_sb, wt, xo

            def emit_vg(wt, xo):
                # Vt = [X Vw^T + Vb | 1] bf16 (bias via ones-row in xo)
                pv = ps_small.tile([128, NT, D], DT.float32, tag="sm")
                for t in range(NT):
                    nc.tensor.matmul(
                        pv[:, t, :],
                        lhsT=xo[:, t * 128 : (t + 1) * 128],
                        rhs=vwb_sb,
                    )
                v_sb = vp.tile([128, NT, 66], DT.bfloat16, tag="v")
                nc.vector.tensor_copy(v_sb[:, :, 0:D], pv)
                nc.vector.memset(v_sb[:, :, D], 1.0)
                # G = O^T W^T [64, 1024] f16, duplicated to partitions 64-127
                g_sb = gp.tile([128, N], DT.float16, tag="g")
                pg = ps_r.tile([D, N], DT.float32, tag="r", name="pg")
                nc.tensor.matmul(
                    pg[:, 0:512], lhsT=o_sb[0:D, :], rhs=wt[0:D, 0:512],
                    tile_position=(0, 0),
                )
                nc.tensor.matmul(
                    pg[:, 512:N], lhsT=o_sb[D : 2 * D, :], rhs=wt[D : 2 * D, 512:N],
                    tile_position=(64, 0),
                )
                nc.scalar.copy(g_sb[0:D, :], pg)
                nc.sync.dma_start(out=g_sb[D : 2 * D, :], in_=g_sb[0:D, :])
                return v_sb, g_sb

            state = {0: emit_loads(0)}
            vg = {0: emit_vg(state[0][1], state[0][2])}

            for b in range(BPC):
                x_sb, wt, xo = state[b]
                v_sb, g_sb = vg[b]

                # ---- per j-tile: R, square, exp, PV ----
                e_sb = ep.tile([128, NT, N], DT.bfloat16, tag="e")
                pot = ps_ot.tile([D + 1, N], DT.float32, tag="ot")
                act_set = {(i * NT) // K_ACT_SQ for i in range(K_ACT_SQ)} if K_ACT_SQ else set()
                for jt in range(NT):
                    pr = ps_r.tile([128, N], DT.float32, tag="r")
                    nc.tensor.matmul(
                        pr[:, 0:512],
                        lhsT=wt[0:D, jt * 128 : (jt + 1) * 128],
                        rhs=g_sb[0:D, 0:512],
                        tile_position=(0, 0),
                    )
                    nc.tensor.matmul(
                        pr[:, 512:N],
                        lhsT=wt[D : 2 * D, jt * 128 : (jt + 1) * 128],
                        rhs=g_sb[D : 2 * D, 512:N],
                        tile_position=(64, 0),
                    )
                    asq = sqp.tile([128, N], DT.float16, tag="asq")
                    if jt in act_set:
                        nc.scalar.activation(asq, pr, AF.Square)
                    else:
                        rf = sqp.tile([128, N], DT.float16, tag="rf")
                        nc.vector.tensor_copy(rf, pr)
                        nc.vector.tensor_mul(asq, rf, rf)
                    nc.scalar.activation(e_sb[:, jt, :], asq, AF.Exp)
                    # PV: OUT^T[65, :] += Vt[jt]^T @ E^T[jt]
                    for c in range(2):
                        nc.tensor.matmul(
                            pot[:, c * 512 : (c + 1) * 512],
                            lhsT=v_sb[:, jt, 0 : D + 1],
                            rhs=e_sb[:, jt, c * 512 : (c + 1) * 512],
                            start=(jt == 0),
                            stop=(jt == NT - 1),
                        )
                    if jt == 2 and b + 1 < BPC:
                        state[b + 1] = emit_loads(b + 1)
                    if jt == 4 and b + 1 < BPC:
                        vg[b + 1] = emit_vg(state[b + 1][1], state[b + 1][2])

                # ---- OUT^T -> SBUF (half DVE, half ACT) ----
                ot_sb = otp.tile([D + 1, N], DT.float32, tag="ot")
                nc.vector.tensor_copy(ot_sb[:, 0:512], pot[:, 0:512])
                nc.scalar.copy(ot_sb[:, 512:N], pot[:, 512:N])

                # ---- transpose back in groups of 4 i-tiles + epilogue ----
                y_sb = yp.tile([128, NT, D], DT.float32, tag="y")
                mean = statp.tile([128, NT], DT.float32, tag="mean")
                var = statp.tile([128, NT], DT.float32, tag="var")
                rstd = statp.tile([128, NT], DT.float32, tag="rstd")
                rcol = statp.tile([128, NT], DT.float32, tag="rcol")
                ysq = statp.tile([128, 4, D], DT.float32, tag="ysq")
                for grp in range(2):
                    ptr = ps_small.tile([128, 4, D + 1], DT.float32, tag="sm")
                    for q in range(4):
                        it = grp * 4 + q
                        nc.tensor.transpose(
                            ptr[:, q, :],
                            ot_sb[:, it * 128 : (it + 1) * 128],
                            id_sb[0 : D + 1, 0 : D + 1],
                        )
                    g_sl = slice(grp * 4, grp * 4 + 4)
                    # 1/rowsum for the 4 tiles at once
                    nc.vector.reciprocal(rcol[:, g_sl], ptr[:, :, D])
                    # y = OUT * (1/rowsum) + x, fused per i-tile
                    for q in range(4):
                        it = grp * 4 + q
                        nc.vector.scalar_tensor_tensor(
                            out=y_sb[:, it, :],
                            in0=ptr[:, q, 0:D],
                            scalar=rcol[:, it : it + 1],
                            in1=x_sb[:, it, :],
                            op0=ALU.mult,
                            op1=ALU.add,
                        )
                    # LN stats via reduces
                    nc.vector.reduce_sum(
                        mean[:, g_sl], y_sb[:, g_sl, :], axis=mybir.AxisListType.X
                    )
                    nc.vector.tensor_mul(ysq, y_sb[:, g_sl, :], y_sb[:, g_sl, :])
                    nc.vector.reduce_sum(
                        var[:, g_sl], ysq, axis=mybir.AxisListType.X
                    )

                # mean/=64; var = var/64 - mean^2 + eps; rstd = rsqrt(var) (Quake+Newton)
                nc.vector.tensor_scalar_mul(mean, mean, 1.0 / D)
                nc.vector.tensor_scalar(
                    var, var, scalar1=1.0 / D, scalar2=1e-5,
                    op0=ALU.mult, op1=ALU.add,
                )
                msq = statp.tile([128, NT], DT.float32, tag="msq")
                nc.gpsimd.tensor_mul(msq, mean, mean)
                nc.gpsimd.tensor_sub(var, var, msq)
                # Quake seed: iv = magic - (i >> 1) == (~(i >> 1)) + (magic + 1)
                iv = statp.tile([128, NT], DT.int32, tag="iv")
                nc.vector.tensor_scalar(
                    iv, var.bitcast(DT.int32), scalar1=1, scalar2=None,
                    op0=ALU.arith_shift_right,
                )
                nc.vector.tensor_scalar(
                    iv, iv, scalar1=-1, scalar2=None, op0=ALU.bitwise_xor
                )
                nc.vector.tensor_scalar_add(iv, iv, 0x5F3759E0)
                yk = iv.bitcast(DT.float32)
                # two Newton steps: y = y * (1.5 - 0.5 v y^2)
                t1 = statp.tile([128, NT], DT.float32, tag="t1")
                for _ in range(2):
                    nc.gpsimd.tensor_mul(t1, yk, yk)
                    nc.gpsimd.tensor_mul(t1, t1, var)
                    nc.vector.tensor_scalar(
                        t1, t1, scalar1=-0.5, scalar2=1.5,
                        op0=ALU.mult, op1=ALU.add,
                    )
                    nc.gpsimd.tensor_mul(yk, yk, t1)
                nc.gpsimd.tensor_copy(rstd, yk)

                for it in range(NT):
                    nc.vector.tensor_scalar(
                        y_sb[:, it, :],
                        y_sb[:, it, :],
                        scalar1=mean[:, it : it + 1],
                        scalar2=rstd[:, it : it + 1],
                        op0=ALU.subtract,
                        op1=ALU.mult,
                    )
                    if apply_gamma_beta:
                        nc.gpsimd.tensor_mul(y_sb[:, it, :], y_sb[:, it, :], gam_sb)
                        nc.gpsimd.tensor_add(y_sb[:, it, :], y_sb[:, it, :], bet_sb)

                nc.sync.dma_start(
                    out=out_t[b].rearrange("(t p) d -> p t d", p=128), in_=y_sb
                )

    nc.compile()
    return nc


_NC_CACHE: dict = {}


def kernel(input1, V_w, V_b, phi, ln_gamma, ln_beta, _trace=False):
    input1 = np.ascontiguousarray(np.asarray(input1, dtype=np.float32))
    V_w = np.asarray(V_w, dtype=np.float32)
    V_b = np.asarray(V_b, dtype=np.float32)
    phi = np.asarray(phi, dtype=np.float32)
    ln_gamma = np.asarray(ln_gamma, dtype=np.float32)
    ln_beta = np.asarray(ln_beta, dtype=np.float32)

    apply_gb = not (np.all(ln_gamma == 1.0) and np.all(ln_beta == 0.0))

    if apply_gb not in _NC_CACHE:
        _NC_CACHE[apply_gb] = _build_nc(apply_gb)
    nc = _NC_CACHE[apply_gb]

    O = _build_orthogonal(phi)
    o16 = np.ascontiguousarray(O.astype(np.float16))
    vwb = np.ascontiguousarray(
        np.concatenate([V_w.T, V_b[None, :]], axis=0).astype(np.float16)
    )
    ident = np.eye(128, dtype=np.float32)
    gb = np.ascontiguousarray(np.stack([ln_gamma, ln_beta]).astype(np.float32))

    # host prep: scaled rows W = x * ||x||^-1/2 * 8^-1/4, transposed; raw X^T + ones row
    ss = (input1.astype(np.float64) ** 2).sum(-1)
    s = (ss ** -0.25 * 8.0 ** -0.25).astype(np.float32)
    w = input1 * s[..., None]
    wt_full = np.ascontiguousarray(w.transpose(0, 2, 1).astype(np.float16))
    xt = input1.transpose(0, 2, 1).astype(np.float16)
    xo_full = np.ascontiguousarray(
        np.concatenate([xt, np.ones((B, 1, N), np.float16)], axis=1)
    )

    in_maps = []
    for c in range(NCORES):
        sl = slice(c * BPC, (c + 1) * BPC)
        in_maps.append(
            {
                "x": np.ascontiguousarray(input1[sl]),
                "wt": wt_full[sl],
                "xo": xo_full[sl],
                "o": o16,
                "vwb": vwb,
                "ident": ident,
                "gb": gb,
            }
        )

    res = bass_utils.run_bass_kernel_spmd(
        nc, in_maps, core_ids=list(range(NCORES)), trace=_trace
    )
    out = np.concatenate([res.results[c]["out"] for c in range(NCORES)], axis=0)
    if _trace:
        kernel._last_result = res
    return out


# revision 19
# speedup vs baseline: 1.2999x; 1.0197x over previous
"""Trainium2 Bass kernel for the hybrid attention head (nn_AttentionHead_Hybrid).

Math (per batch):
    norms  n_i = ||x_i||;  xh = x / n
    O      = product of 2016 Givens rotations (built on host, fp32)
    S[i,j] = xh_i . O . xh_j
    A      = S^2 * n_i n_j ;  P = softmax(A / 8)
    V      = x @ Vw^T + Vb
    out    = LayerNorm(P @ V + x) * gamma + beta

Device formulation (per core, 4 batches):
    W      = diag(s) X with s_n = ||x_n||^-1/2 * 8^-1/4   (host-prepped, f16, transposed)
    R      = W O^T W^T          ->  R[j,i]^2 = A[i,j]/8
    E^T[j,i] = exp(R^2)         (bf16)
    Vt     = [X Vw^T + Vb | 1]  (bf16; ones column gives softmax row-sums for free,
                                 bias via ones-row appended to X^T on host)
    OUT^T  = sum_j Vt[j,:]^T E^T[j,:]   in [65, N] psum, then PE-transpose back
    attn   = OUT[:, :64] / OUT[:, 64];  out = LN(attn + x)

Sharding: data-parallel over batch, 4 batches per core on 8 cores; params replicated.
"""

import math

import numpy as np

import concourse.bacc as bacc
import concourse.bass as bass
import concourse.tile as tile
from concourse import bass_utils, mybir

AF = mybir.ActivationFunctionType
ALU = mybir.AluOpType
DT = mybir.dt

B, N, D = 32, 1024, 64
NCORES = 8
BPC = B // NCORES          # batches per core
NT = N // 128              # 128-row tiles per batch

# how many of the 8 per-batch [128,1024] square chunks run on ACT (rest on DVE)
K_ACT_SQ = 5


def _build_orthogonal(phi: np.ndarray, d: int = D) -> np.ndarray:
    """fp32 replica of the reference jax.lax.scan Givens chain."""
    O = np.eye(d, dtype=np.float32)
    ii, jj = np.triu_indices(d, k=1)
    c = np.cos(phi.astype(np.float32))
    s = np.sin(phi.astype(np.float32))
    for k in range(len(phi)):
        i, j = int(ii[k]), int(jj[k])
        ri = O[i].copy()
        rj = O[j].copy()
        O[i] = c[k] * ri + s[k] * rj
        O[j] = -s[k] * ri + c[k] * rj
    return O


def _build_nc(apply_gamma_beta: bool):
    nc = bacc.Bacc("TRN2", target_bir_lowering=False)

    x_t = nc.dram_tensor("x", [BPC, N, D], DT.float32, kind="ExternalInput")
    wt_t = nc.dram_tensor("wt", [BPC, D, N], DT.float16, kind="ExternalInput")
    xo_t = nc.dram_tensor("xo", [BPC, D + 1, N], DT.float16, kind="ExternalInput")
    o_t = nc.dram_tensor("o", [D, D], DT.float16, kind="ExternalInput")
    vwb_t = nc.dram_tensor("vwb", [D + 1, D], DT.float16, kind="ExternalInput")
    id_t = nc.dram_tensor("ident", [128, 128], DT.float32, kind="ExternalInput")
    gb_t = nc.dram_tensor("gb", [2, D], DT.float32, kind="ExternalInput")
    out_t = nc.dram_tensor("out", [BPC, N, D], DT.float32, kind="ExternalOutput")

    with tile.TileContext(nc) as tc:
        with (
            tc.tile_pool(name="const", bufs=1) as constp,
            tc.tile_pool(name="xp", bufs=2) as xp,
            tc.tile_pool(name="wtp", bufs=2) as wtp,
            tc.tile_pool(name="gp", bufs=2) as gp,
            tc.tile_pool(name="ep", bufs=2) as ep,
            tc.tile_pool(name="vp", bufs=2) as vp,
            tc.tile_pool(name="sqp", bufs=4) as sqp,
            tc.tile_pool(name="otp", bufs=2) as otp,
            tc.tile_pool(name="yp", bufs=2) as yp,
            tc.tile_pool(name="statp", bufs=2) as statp,
            tc.tile_pool(name="ps_r", bufs=2, space="PSUM") as ps_r,
            tc.tile_pool(name="ps_ot", bufs=1, space="PSUM") as ps_ot,
            tc.tile_pool(name="ps_small", bufs=2, space="PSUM") as ps_small,
        ):
            o_sb = constp.tile([128, D], DT.float16)
            nc.sync.dma_start(out=o_sb[0:D, :], in_=o_t[:, :])
            nc.sync.dma_start(out=o_sb[D : 2 * D, :], in_=o_t[:, :])
            vwb_sb = constp.tile([D + 1, D], DT.float16)
            nc.sync.dma_start(out=vwb_sb, in_=vwb_t[:, :])
            id_sb = constp.tile([128, 128], DT.float32)
            nc.sync.dma_start(out=id_sb, in_=id_t[:, :])
            if apply_gamma_beta:
                gam_sb = constp.tile([128, D], DT.float32)
                nc.sync.dma_start(out=gam_sb, in_=gb_t[0, :].to_broadcast([128, D]))
                bet_sb = constp.tile([128, D], DT.float32)
                nc.sync.dma_start(out=bet_sb, in_=gb_t[1, :].to_broadcast([128, D]))

            # PE warm-up: ~12 dependency-free matmuls trip the HAM
            # activity window so real matmuls run at 2.4 GHz, not 1.2.
            warm = constp.tile([64, 512], DT.float16)
            nc.vector.memset(warm, 0.0)
            pw = ps_small.tile([64, 512], DT.float32, tag="sm")
            for _ in range(8):
                nc.tensor.matmul(pw, lhsT=warm[:, 0:64], rhs=warm)

            def emit_loads(b):
                x_sb = xp.tile([128, NT, D], DT.float32, tag="x")
                nc.sync.dma_start(
                    out=x_sb, in_=x_t[b].rearrange("(t p) d -> p t d", p=128)
                )
                wt = wtp.tile([128, N], DT.float16, tag="wt")
                nc.sync.dma_start(out=wt[0:D, :], in_=wt_t[b])
                nc.sync.dma_start(out=wt[D : 2 * D, :], in_=wt_t[b])
                xo = wtp.tile([D + 1, N], DT.float16, tag="xo")
                nc.sync.dma_start(out=xo, in_=xo_t[b])
                return x_sb, wt, xo

            def emit_vg(wt, xo):
                # Vt = [X Vw^T + Vb | 1] bf16 (bias via ones-row in xo)
                pv = ps_small.tile([128, NT, D], DT.float32, tag="sm")
                for t in range(NT):
                    nc.tensor.matmul(
                        pv[:, t, :],
                        lhsT=xo[:, t * 128 : (t + 1) * 128],
                        rhs=vwb_sb,
                    )
                v_sb = vp.tile([128, NT, 66], DT.bfloat16, tag="v")
                nc.vector.tensor_copy(v_sb[:, :, 0:D], pv)
                nc.vector.memset(v_sb[:, :, D], 1.0)
                # G = O^T W^T [64, 1024] f16, duplicated to partitions 64-127
                g_sb = gp.tile([128, N], DT.float16, tag="g")
                pg = ps_r.tile([D, N], DT.float32, tag="r", name="pg")
                nc.tensor.matmul(
                    pg[:, 0:512], lhsT=o_sb[0:D, :], rhs=wt[0:D, 0:512],
                    tile_position=(0, 0),
                )
                nc.tensor.matmul(
                    pg[:, 512:N], lhsT=o_sb[D : 2 * D, :], rhs=wt[D : 2 * D, 512:N],
                    tile_position=(64, 0),
                )
                nc.scalar.copy(g_sb[0:D, :], pg)
                nc.sync.dma_start(out=g_sb[D : 2 * D, :], in_=g_sb[0:D, :])
                return v_sb, g_sb

            state = {0: emit_loads(0)}
            vg = {0: emit_vg(state[0][1], state[0][2])}

            for b in range(BPC):
                x_sb, wt, xo = state[b]
                v_sb, g_sb = vg[b]

                # ---- per j-tile: R, square, exp, PV ----
                e_sb = ep.tile([128, NT, N], DT.bfloat16, tag="e")
                pot = ps_ot.tile([D + 1, N], DT.float32, tag="ot")
                act_set = {(i * NT) // K_ACT_SQ for i in range(K_ACT_SQ)} if K_ACT_SQ else set()
                for jt in range(NT):
                    pr = ps_r.tile([128, N], DT.float32, tag="r")
                    nc.tensor.matmul(
                        pr[:, 0:512],
                        lhsT=wt[0:D, jt * 128 : (jt + 1) * 128],
                        rhs=g_sb[0:D, 0:512],
                        tile_position=(0, 0),
                    )
                    nc.tensor.matmul(
                        pr[:, 512:N],
                        lhsT=wt[D : 2 * D, jt * 128 : (jt + 1) * 128],
                        rhs=g_sb[D : 2 * D, 512:N],
                        tile_position=(64, 0),
                    )
                    asq = sqp.tile([128, N], DT.float16, tag="asq")
                    if jt in act_set:
                        nc.scalar.activation(asq, pr, AF.Square)
                    else:
                        rf = sqp.tile([128, N], DT.float16, tag="rf")
                        nc.vector.tensor_copy(rf, pr)
                        nc.vector.tensor_mul(asq, rf, rf)
                    nc.scalar.activation(e_sb[:, jt, :], asq, AF.Exp)
                    # PV: OUT^T[65, :] += Vt[jt]^T @ E^T[jt]
                    for c in range(2):
                        nc.tensor.matmul(
                            pot[:, c * 512 : (c + 1) * 512],
                            lhsT=v_sb[:, jt, 0 : D + 1],
                            rhs=e_sb[:, jt, c * 512 : (c + 1) * 512],
                            start=(jt == 0),
                            stop=(jt == NT - 1),
                        )
                    if jt == 2 and b + 1 < BPC:
                        state[b + 1] = emit_loads(b + 1)
                    if jt == 4 and b + 1 < BPC:
                        vg[b + 1] = emit_vg(state[b + 1][1], state[b + 1][2])

                # ---- OUT^T -> SBUF (half DVE, half ACT) ----
                ot_sb = otp.tile([D + 1, N], DT.float32, tag="ot")
                nc.vector.tensor_copy(ot_sb[:, 0:512], pot[:, 0:512])
                nc.scalar.copy(ot_sb[:, 512:N], pot[:, 512:N])

                # ---- transpose back in groups of 4 i-tiles + epilogue ----
                y_sb = yp.tile([128, NT, D], DT.float32, tag="y")
                mean = statp.tile([128, NT], DT.float32, tag="mean")
                var = statp.tile([128, NT], DT.float32, tag="var")
                rstd = statp.tile([128, NT], DT.float32, tag="rstd")
                rcol = statp.tile([128, NT], DT.float32, tag="rcol")
                ysq = statp.tile([128, 4, D], DT.float32, tag="ysq")
                for grp in range(2):
                    ptr = ps_small.tile([128, 4, D + 1], DT.float32, tag="sm")
                    for q in range(4):
                        it = grp * 4 + q
                        nc.tensor.transpose(
                            ptr[:, q, :],
                            ot_sb[:, it * 128 : (it + 1) * 128],
                            id_sb[0 : D + 1, 0 : D + 1],
                        )
                    g_sl = slice(grp * 4, grp * 4 + 4)
                    # 1/rowsum for the 4 tiles at once
                    nc.vector.reciprocal(rcol[:, g_sl], ptr[:, :, D])
                    # y = OUT * (1/rowsum) + x, fused per i-tile
                    for q in range(4):
                        it = grp * 4 + q
                        nc.vector.scalar_tensor_tensor(
                            out=y_sb[:, it, :],
                            in0=ptr[:, q, 0:D],
                            scalar=rcol[:, it : it + 1],
                            in1=x_sb[:, it, :],
                            op0=ALU.mult,
                            op1=ALU.add,
                        )
                    # LN stats via reduces
                    nc.vector.reduce_sum(
                        mean[:, g_sl], y_sb[:, g_sl, :], axis=mybir.AxisListType.X
                    )
                    nc.vector.tensor_mul(ysq, y_sb[:, g_sl, :], y_sb[:, g_sl, :])
                    nc.vector.reduce_sum(
                        var[:, g_sl], ysq, axis=mybir.AxisListType.X
                    )

                # mean/=64; var = var/64 - mean^2 + eps; rstd = rsqrt(var) (Quake+Newton)
                nc.vector.tensor_scalar_mul(mean, mean, 1.0 / D)
                nc.vector.tensor_scalar(
                    var, var, scalar1=1.0 / D, scalar2=1e-5,
                    op0=ALU.mult, op1=ALU.add,
                )
                msq = statp.tile([128, NT], DT.float32, tag="msq")
                nc.gpsimd.tensor_mul(msq, mean, mean)
                nc.gpsimd.tensor_sub(var, var, msq)
                # Quake seed: iv = magic - (i >> 1) == (~(i >> 1)) + (magic + 1)
                iv = statp.tile([128, NT], DT.int32, tag="iv")
                nc.vector.tensor_scalar(
                    iv, var.bitcast(DT.int32), scalar1=1, scalar2=None,
                    op0=ALU.arith_shift_right,
                )
                nc.vector.tensor_scalar(
                    iv, iv, scalar1=-1, scalar2=None, op0=ALU.bitwise_xor
                )
                nc.vector.tensor_scalar_add(iv, iv, 0x5F3759E0)
                yk = iv.bitcast(DT.float32)
                # two Newton steps: y = y * (1.5 - 0.5 v y^2)
                t1 = statp.tile([128, NT], DT.float32, tag="t1")
                for _ in range(1):
                    nc.gpsimd.tensor_mul(t1, yk, yk)
                    nc.gpsimd.tensor_mul(t1, t1, var)
                    nc.vector.tensor_scalar(
                        t1, t1, scalar1=-0.5, scalar2=1.5,
                        op0=ALU.mult, op1=ALU.add,
                    )
                    nc.gpsimd.tensor_mul(yk, yk, t1)
                nc.gpsimd.tensor_copy(rstd, yk)

                mean_bc = bass.AP(
                    tensor=mean.tensor, offset=mean.offset,
                    ap=[mean.ap[0], [1, NT], [0, D]],
                )
                rstd_bc = bass.AP(
                    tensor=rstd.tensor, offset=rstd.offset,
                    ap=[rstd.ap[0], [1, NT], [0, D]],
                )
                nc.vector.tensor_tensor(out=y_sb, in0=y_sb, in1=mean_bc, op=ALU.subtract)
                nc.vector.tensor_tensor(out=y_sb, in0=y_sb, in1=rstd_bc, op=ALU.mult)
                if apply_gamma_beta:
                    for it in range(NT):
                        nc.gpsimd.tensor_mul(y_sb[:, it, :], y_sb[:, it, :], gam_sb)
                        nc.gpsimd.tensor_add(y_sb[:, it, :], y_sb[:, it, :], bet_sb)

                nc.sync.dma_start(
                    out=out_t[b].rearrange("(t p) d -> p t d", p=128), in_=y_sb
                )

    nc.compile()
    return nc


_NC_CACHE: dict = {}


def kernel(input1, V_w, V_b, phi, ln_gamma, ln_beta, _trace=False):
    input1 = np.ascontiguousarray(np.asarray(input1, dtype=np.float32))
    V_w = np.asarray(V_w, dtype=np.float32)
    V_b = np.asarray(V_b, dtype=np.float32)
    phi = np.asarray(phi, dtype=np.float32)
    ln_gamma = np.asarray(ln_gamma, dtype=np.float32)
    ln_beta = np.asarray(ln_beta, dtype=np.float32)

    apply_gb = not (np.all(ln_gamma == 1.0) and np.all(ln_beta == 0.0))

    if apply_gb not in _NC_CACHE:
        _NC_CACHE[apply_gb] = _build_nc(apply_gb)
    nc = _NC_CACHE[apply_gb]

    O = _build_orthogonal(phi)
    o16 = np.ascontiguousarray(O.astype(np.float16))
    vwb = np.ascontiguousarray(
        np.concatenate([V_w.T, V_b[None, :]], axis=0).astype(np.float16)
    )
    ident = np.eye(128, dtype=np.float32)
    gb = np.ascontiguousarray(np.stack([ln_gamma, ln_beta]).astype(np.float32))

    # host prep: scaled rows W = x * ||x||^-1/2 * 8^-1/4, transposed; raw X^T + ones row
    ss = (input1.astype(np.float64) ** 2).sum(-1)
    s = (ss ** -0.25 * 8.0 ** -0.25).astype(np.float32)
    w = input1 * s[..., None]
    wt_full = np.ascontiguousarray(w.transpose(0, 2, 1).astype(np.float16))
    xt = input1.transpose(0, 2, 1).astype(np.float16)
    xo_full = np.ascontiguousarray(
        np.concatenate([xt, np.ones((B, 1, N), np.float16)], axis=1)
    )

    in_maps = []
    for c in range(NCORES):
        sl = slice(c * BPC, (c + 1) * BPC)
        in_maps.append(
            {
                "x": np.ascontiguousarray(input1[sl]),
                "wt": wt_full[sl],
                "xo": xo_full[sl],
                "o": o16,
                "vwb": vwb,
                "ident": ident,
                "gb": gb,
            }
        )

    res = bass_utils.run_bass_kernel_spmd(
        nc, in_maps, core_ids=list(range(NCORES)), trace=_trace
    )
    out = np.concatenate([res.results[c]["out"] for c in range(NCORES)], axis=0)
    if _trace:
        kernel._last_result = res
    return out


# revision 20
# speedup vs baseline: 1.3509x; 1.0392x over previous
"""Trainium2 Bass kernel for the hybrid attention head (nn_AttentionHead_Hybrid).

Math (per batch):
    norms  n_i = ||x_i||;  xh = x / n
    O      = product of 2016 Givens rotations (built on host, fp32)
    S[i,j] = xh_i . O . xh_j
    A      = S^2 * n_i n_j ;  P = softmax(A / 8)
    V      = x @ Vw^T + Vb
    out    = LayerNorm(P @ V + x) * gamma + beta

Device formulation (per core, 4 batches):
    W      = diag(s) X with s_n = ||x_n||^-1/2 * 8^-1/4   (host-prepped, f16, transposed)
    R      = W O^T W^T          ->  R[j,i]^2 = A[i,j]/8
    E^T[j,i] = exp(R^2)         (bf16)
    Vt     = [X Vw^T + Vb | 1]  (bf16; ones column gives softmax row-sums for free,
                                 bias via ones-row appended to X^T on host)
    OUT^T  = sum_j Vt[j,:]^T E^T[j,:]   in [65, N] psum, then PE-transpose back
    attn   = OUT[:, :64] / OUT[:, 64];  out = LN(attn + x)

Sharding: data-parallel over batch, 4 batches per core on 8 cores; params replicated.
"""

import math

import numpy as np

import concourse.bacc as bacc
import concourse.bass as bass
import concourse.tile as tile
from concourse import bass_utils, mybir

AF = mybir.ActivationFunctionType
ALU = mybir.AluOpType
DT = mybir.dt

B, N, D = 32, 1024, 64
NCORES = 8
BPC = B // NCORES          # batches per core
NT = N // 128              # 128-row tiles per batch

# how many of the 8 per-batch [128,1024] square chunks run on ACT (rest on DVE)
K_ACT_SQ = 4


def _build_orthogonal(phi: np.ndarray, d: int = D) -> np.ndarray:
    """fp32 replica of the reference jax.lax.scan Givens chain."""
    O = np.eye(d, dtype=np.float32)
    ii, jj = np.triu_indices(d, k=1)
    c = np.cos(phi.astype(np.float32))
    s = np.sin(phi.astype(np.float32))
    for k in range(len(phi)):
        i, j = int(ii[k]), int(jj[k])
        ri = O[i].copy()
        rj = O[j].copy()
        O[i] = c[k] * ri + s[k] * rj
        O[j] = -s[k] * ri + c[k] * rj
    return O


def _build_nc(apply_gamma_beta: bool):
    nc = bacc.Bacc("TRN2", target_bir_lowering=False)

    x_t = nc.dram_tensor("x", [BPC, N, D], DT.float32, kind="ExternalInput")
    wt_t = nc.dram_tensor("wt", [BPC, D, N], DT.float16, kind="ExternalInput")
    xo_t = nc.dram_tensor("xo", [BPC, D + 1, N], DT.float16, kind="ExternalInput")
    o_t = nc.dram_tensor("o", [D, D], DT.float16, kind="ExternalInput")
    vwb_t = nc.dram_tensor("vwb", [D + 1, D], DT.float16, kind="ExternalInput")
    id_t = nc.dram_tensor("ident", [128, 128], DT.float32, kind="ExternalInput")
    gb_t = nc.dram_tensor("gb", [2, D], DT.float32, kind="ExternalInput")
    out_t = nc.dram_tensor("out", [BPC, N, D], DT.float32, kind="ExternalOutput")

    with tile.TileContext(nc) as tc:
        with (
            tc.tile_pool(name="const", bufs=1) as constp,
            tc.tile_pool(name="xp", bufs=2) as xp,
            tc.tile_pool(name="wtp", bufs=2) as wtp,
            tc.tile_pool(name="gp", bufs=2) as gp,
            tc.tile_pool(name="ep", bufs=2) as ep,
            tc.tile_pool(name="vp", bufs=2) as vp,
            tc.tile_pool(name="sqp", bufs=4) as sqp,
            tc.tile_pool(name="otp", bufs=2) as otp,
            tc.tile_pool(name="yp", bufs=2) as yp,
            tc.tile_pool(name="statp", bufs=2) as statp,
            tc.tile_pool(name="ps_r", bufs=2, space="PSUM") as ps_r,
            tc.tile_pool(name="ps_ot", bufs=1, space="PSUM") as ps_ot,
            tc.tile_pool(name="ps_small", bufs=2, space="PSUM") as ps_small,
        ):
            o_sb = constp.tile([128, D], DT.float16)
            nc.sync.dma_start(out=o_sb[0:D, :], in_=o_t[:, :])
            nc.sync.dma_start(out=o_sb[D : 2 * D, :], in_=o_t[:, :])
            vwb_sb = constp.tile([D + 1, D], DT.float16)
            nc.sync.dma_start(out=vwb_sb, in_=vwb_t[:, :])
            id_sb = constp.tile([128, 128], DT.float32)
            nc.sync.dma_start(out=id_sb, in_=id_t[:, :])
            if apply_gamma_beta:
                gam_sb = constp.tile([128, D], DT.float32)
                nc.sync.dma_start(out=gam_sb, in_=gb_t[0, :].to_broadcast([128, D]))
                bet_sb = constp.tile([128, D], DT.float32)
                nc.sync.dma_start(out=bet_sb, in_=gb_t[1, :].to_broadcast([128, D]))

            # PE warm-up: ~12 dependency-free matmuls trip the HAM
            # activity window so real matmuls run at 2.4 GHz, not 1.2.
            warm = constp.tile([64, 512], DT.float16)
            nc.vector.memset(warm, 0.0)
            pw = ps_small.tile([64, 512], DT.float32, tag="sm")
            for _ in range(8):
                nc.tensor.matmul(pw, lhsT=warm[:, 0:64], rhs=warm)

            def emit_loads(b):
                wt = wtp.tile([128, N], DT.float16, tag="wt")
                nc.sync.dma_start(out=wt[0:D, :], in_=wt_t[b])
                nc.sync.dma_start(out=wt[D : 2 * D, :], in_=wt_t[b])
                xo = wtp.tile([D + 1, N], DT.float16, tag="xo")
                nc.sync.dma_start(out=xo, in_=xo_t[b])
                x_sb = xp.tile([128, NT, D], DT.float32, tag="x")
                nc.sync.dma_start(
                    out=x_sb, in_=x_t[b].rearrange("(t p) d -> p t d", p=128)
                )
                return x_sb, wt, xo

            def emit_vg(wt, xo):
                # G = O^T W^T [64, 1024] f16 (chunk pair via PE row groups),
                # duplicated to partitions 64-127
                g_sb = gp.tile([128, N], DT.float16, tag="g")
                pg = ps_r.tile([D, N], DT.float32, tag="r", name="pg")
                nc.tensor.matmul(
                    pg[:, 0:512], lhsT=o_sb[0:D, :], rhs=wt[0:D, 0:512],
                    tile_position=(0, 0),
                )
                nc.tensor.matmul(
                    pg[:, 512:N], lhsT=o_sb[D : 2 * D, :], rhs=wt[D : 2 * D, 512:N],
                    tile_position=(64, 0),
                )
                nc.scalar.copy(g_sb[0:D, :], pg)
                nc.sync.dma_start(out=g_sb[D : 2 * D, :], in_=g_sb[0:D, :])
                # Vt = [X Vw^T + Vb | 1] bf16 (bias via ones-row in xo)
                pv = ps_small.tile([128, NT, D], DT.float32, tag="sm")
                for t in range(NT):
                    nc.tensor.matmul(
                        pv[:, t, :],
                        lhsT=xo[:, t * 128 : (t + 1) * 128],
                        rhs=vwb_sb,
                    )
                v_sb = vp.tile([128, NT, 66], DT.bfloat16, tag="v")
                nc.vector.tensor_copy(v_sb[:, :, 0:D], pv)
                nc.vector.memset(v_sb[:, :, D], 1.0)
                return v_sb, g_sb

            state = {0: emit_loads(0)}
            vg = {0: emit_vg(state[0][1], state[0][2])}

            for b in range(BPC):
                x_sb, wt, xo = state[b]
                v_sb, g_sb = vg[b]

                # ---- per j-tile: R, square, exp, PV ----
                e_sb = ep.tile([128, NT, N], DT.bfloat16, tag="e")
                pot = ps_ot.tile([D + 1, N], DT.float32, tag="ot")
                act_set = {(i * NT) // K_ACT_SQ for i in range(K_ACT_SQ)} if K_ACT_SQ else set()
                for jt in range(NT):
                    pr = ps_r.tile([128, N], DT.float32, tag="r")
                    nc.tensor.matmul(
                        pr[:, 0:512],
                        lhsT=wt[0:D, jt * 128 : (jt + 1) * 128],
                        rhs=g_sb[0:D, 0:512],
                        tile_position=(0, 0),
                    )
                    nc.tensor.matmul(
                        pr[:, 512:N],
                        lhsT=wt[D : 2 * D, jt * 128 : (jt + 1) * 128],
                        rhs=g_sb[D : 2 * D, 512:N],
                        tile_position=(64, 0),
                    )
                    asq = sqp.tile([128, N], DT.float16, tag="asq")
                    if jt in act_set:
                        nc.scalar.activation(asq, pr, AF.Square)
                    else:
                        rf = sqp.tile([128, N], DT.float16, tag="rf")
                        nc.vector.tensor_copy(rf, pr)
                        nc.vector.tensor_mul(asq, rf, rf)
                    nc.scalar.activation(e_sb[:, jt, :], asq, AF.Exp)
                    # PV: OUT^T[65, :] += Vt[jt]^T @ E^T[jt]
                    for c in range(2):
                        nc.tensor.matmul(
                            pot[:, c * 512 : (c + 1) * 512],
                            lhsT=v_sb[:, jt, 0 : D + 1],
                            rhs=e_sb[:, jt, c * 512 : (c + 1) * 512],
                            start=(jt == 0),
                            stop=(jt == NT - 1),
                        )
                    if jt == 2 and b + 1 < BPC:
                        state[b + 1] = emit_loads(b + 1)
                    if jt == 4 and b + 1 < BPC:
                        vg[b + 1] = emit_vg(state[b + 1][1], state[b + 1][2])

                # ---- OUT^T -> SBUF (half DVE, half ACT) ----
                ot_sb = otp.tile([D + 1, N], DT.float32, tag="ot")
                nc.vector.tensor_copy(ot_sb[:, 0:512], pot[:, 0:512])
                nc.scalar.copy(ot_sb[:, 512:N], pot[:, 512:N])

                # ---- transpose back in groups of 4 i-tiles + epilogue ----
                y_sb = yp.tile([128, NT, D], DT.float32, tag="y")
                mean = statp.tile([128, NT], DT.float32, tag="mean")
                var = statp.tile([128, NT], DT.float32, tag="var")
                rstd = statp.tile([128, NT], DT.float32, tag="rstd")
                rcol = statp.tile([128, NT], DT.float32, tag="rcol")
                ysq = statp.tile([128, 4, D], DT.float32, tag="ysq")
                for grp in range(2):
                    ptr = ps_small.tile([128, 4, D + 1], DT.float32, tag="sm")
                    for q in range(4):
                        it = grp * 4 + q
                        nc.tensor.transpose(
                            ptr[:, q, :],
                            ot_sb[:, it * 128 : (it + 1) * 128],
                            id_sb[0 : D + 1, 0 : D + 1],
                        )
                    g_sl = slice(grp * 4, grp * 4 + 4)
                    # 1/rowsum for the 4 tiles at once
                    nc.vector.reciprocal(rcol[:, g_sl], ptr[:, :, D])
                    # y = OUT * (1/rowsum) + x (batched over the 4 tiles)
                    rc4 = rcol[:, g_sl]
                    rcol_bc = bass.AP(
                        tensor=rcol.tensor, offset=rc4.offset,
                        ap=[rcol.ap[0], [1, 4], [0, D]],
                    )
                    nc.vector.tensor_tensor(
                        out=y_sb[:, g_sl, :], in0=ptr[:, :, 0:D], in1=rcol_bc,
                        op=ALU.mult,
                    )
                    nc.vector.tensor_add(
                        y_sb[:, g_sl, :], y_sb[:, g_sl, :], x_sb[:, g_sl, :]
                    )
                    # LN stats via reduces
                    nc.vector.reduce_sum(
                        mean[:, g_sl], y_sb[:, g_sl, :], axis=mybir.AxisListType.X
                    )
                    nc.vector.tensor_mul(ysq, y_sb[:, g_sl, :], y_sb[:, g_sl, :])
                    nc.vector.reduce_sum(
                        var[:, g_sl], ysq, axis=mybir.AxisListType.X
                    )

                # mean/=64; var = var/64 - mean^2 + eps; rstd = rsqrt(var) (Quake+Newton)
                nc.vector.tensor_scalar_mul(mean, mean, 1.0 / D)
                nc.vector.tensor_scalar(
                    var, var, scalar1=1.0 / D, scalar2=1e-5,
                    op0=ALU.mult, op1=ALU.add,
                )
                msq = statp.tile([128, NT], DT.float32, tag="msq")
                nc.gpsimd.tensor_mul(msq, mean, mean)
                nc.gpsimd.tensor_sub(var, var, msq)
                # Quake seed: iv = magic - (i >> 1) == (~(i >> 1)) + (magic + 1)
                iv = statp.tile([128, NT], DT.int32, tag="iv")
                nc.vector.tensor_scalar(
                    iv, var.bitcast(DT.int32), scalar1=1, scalar2=None,
                    op0=ALU.arith_shift_right,
                )
                nc.vector.tensor_scalar(
                    iv, iv, scalar1=-1, scalar2=None, op0=ALU.bitwise_xor
                )
                nc.vector.tensor_scalar_add(iv, iv, 0x5F3759E0)
                yk = iv.bitcast(DT.float32)
                # two Newton steps: y = y * (1.5 - 0.5 v y^2)
                t1 = statp.tile([128, NT], DT.float32, tag="t1")
                for _ in range(1):
                    nc.gpsimd.tensor_mul(t1, yk, yk)
                    nc.gpsimd.tensor_mul(t1, t1, var)
                    nc.vector.tensor_scalar(
                        t1, t1, scalar1=-0.5, scalar2=1.5,
                        op0=ALU.mult, op1=ALU.add,
                    )
                    nc.gpsimd.tensor_mul(yk, yk, t1)
                nc.gpsimd.tensor_copy(rstd, yk)

                mean_bc = bass.AP(
                    tensor=mean.tensor, offset=mean.offset,
                    ap=[mean.ap[0], [1, NT], [0, D]],
                )
                rstd_bc = bass.AP(
                    tensor=rstd.tensor, offset=rstd.offset,
                    ap=[rstd.ap[0], [1, NT], [0, D]],
                )
                nc.vector.tensor_tensor(out=y_sb, in0=y_sb, in1=mean_bc, op=ALU.subtract)
                nc.vector.tensor_tensor(out=y_sb, in0=y_sb, in1=rstd_bc, op=ALU.mult)
                if apply_gamma_beta:
                    for it in range(NT):
                        nc.gpsimd.tensor_mul(y_sb[:, it, :], y_sb[:, it, :], gam_sb)
                        nc.gpsimd.tensor_add(y_sb[:, it, :], y_sb[:, it, :], bet_sb)

                nc.sync.dma_start(
                    out=out_t[b].rearrange("(t p) d -> p t d", p=128), in_=y_sb
                )

    nc.compile()
    return nc


_NC_CACHE: dict = {}


def kernel(input1, V_w, V_b, phi, ln_gamma, ln_beta, _trace=False):
    input1 = np.ascontiguousarray(np.asarray(input1, dtype=np.float32))
    V_w = np.asarray(V_w, dtype=np.float32)
    V_b = np.asarray(V_b, dtype=np.float32)
    phi = np.asarray(phi, dtype=np.float32)
    ln_gamma = np.asarray(ln_gamma, dtype=np.float32)
    ln_beta = np.asarray(ln_beta, dtype=np.float32)

    apply_gb = not (np.all(ln_gamma == 1.0) and np.all(ln_beta == 0.0))

    if apply_gb not in _NC_CACHE:
        _NC_CACHE[apply_gb] = _build_nc(apply_gb)
    nc = _NC_CACHE[apply_gb]

    O = _build_orthogonal(phi)
    o16 = np.ascontiguousarray(O.astype(np.float16))
    vwb = np.ascontiguousarray(
        np.concatenate([V_w.T, V_b[None, :]], axis=0).astype(np.float16)
    )
    ident = np.eye(128, dtype=np.float32)
    gb = np.ascontiguousarray(np.stack([ln_gamma, ln_beta]).astype(np.float32))

    # host prep: scaled rows W = x * ||x||^-1/2 * 8^-1/4, transposed; raw X^T + ones row
    ss = (input1.astype(np.float64) ** 2).sum(-1)
    s = (ss ** -0.25 * 8.0 ** -0.25).astype(np.float32)
    w = input1 * s[..., None]
    wt_full = np.ascontiguousarray(w.transpose(0, 2, 1).astype(np.float16))
    xt = input1.transpose(0, 2, 1).astype(np.float16)
    xo_full = np.ascontiguousarray(
        np.concatenate([xt, np.ones((B, 1, N), np.float16)], axis=1)
    )

    in_maps = []
    for c in range(NCORES):
        sl = slice(c * BPC, (c + 1) * BPC)
        in_maps.append(
            {
                "x": np.ascontiguousarray(input1[sl]),
                "wt": wt_full[sl],
                "xo": xo_full[sl],
                "o": o16,
                "vwb": vwb,
                "ident": ident,
                "gb": gb,
            }
        )

    res = bass_utils.run_bass_kernel_spmd(
        nc, in_maps, core_ids=list(range(NCORES)), trace=_trace
    )
    out = np.concatenate([res.results[c]["out"] for c in range(NCORES)], axis=0)
    if _trace:
        kernel._last_result = res
    return out
